# revision 16
# baseline (speedup 1.0000x reference)
"""Trainium2 Bass kernel for a multi-head-attention block (B,C,N,D = 8,4,1024,96;
H=3 heads, dk=dv=32; softmax over the QUERY axis; residual + LayerNorm).

Sharding: pure data-parallel over batch B across 8 NeuronCores (one batch
element per core, C=4 channel-slices each, no collectives).

v7 (baseline 188us -> v5 175us):
  - ScalarE is the pacing engine (96 exp instructions over 12.6M scores +
    accumulator reads ~= 118us); everything else is scheduled into its
    shadow. NOTE: keeping the PE densely busy also keeps the chip power
    state up - an under-loaded PE measurably slows the ACT engine's exp
    throughput (~19%), so aux work stays on-device on the PE.
  - PSUM: 3-deep score ring (6 banks) + ctx accumulator (2 banks). Aux work
    (transposes / projections / fc) shares the score ring but is merged into
    at most one allocation per chunk-slot so the ring's exp->scores
    self-pacing is never disturbed and PE gaps stay short.
  - input transposes use the PE transpose instruction (f32r 1.5 cycles/row
    vs 4 for a sub-256-free f32r matmul); V projection and fc run in bf16
    (1 cycle/row); h-outer emission halves LDWEIGHTS traffic.
  - denominator reciprocal + V-rescale for chunk i on DVE at the TOP of
    slot i+1 (before the slot's PE-gated casts) so ctx never waits.
  - prologue: input DMAs interleaved across the SP and Activation queues at
    half-tensor granularity; V path runs inside the first exp-block shadow.
  - drain: last channel's fc/LN/store pipelined at half-channel granularity
    with a split output DMA.
"""

from contextlib import ExitStack

import numpy as np

import concourse.bass as bass
import concourse.tile as tile
from concourse import bacc, mybir
from concourse.bass_utils import run_bass_kernel_spmd

F32 = mybir.dt.float32
BF16 = mybir.dt.bfloat16
F32R = mybir.dt.float32r
I32 = mybir.dt.int32
A = mybir.AluOpType

B, C, N, D = 8, 4, 1024, 96
H, DK, DV = 3, 32, 32
P = 128               # partition size / token chunk
NCHUNK = N // P       # 8
QT = 512              # matmul free-dim limit into one PSUM bank (f32)
SCALE = 1.0 / np.sqrt(DK)
EPS = 1e-5

_CACHE = {}


def _emit(nc, tc, ctx, apply_affine):
    xq_d = nc.dram_tensor("xq", [C, N, D], F32R, kind="ExternalInput").ap()
    xk_d = nc.dram_tensor("xk", [C, N, D], F32R, kind="ExternalInput").ap()
    xv_d = nc.dram_tensor("xv", [C, N, D], F32R, kind="ExternalInput").ap()
    # wall = host-packed [128, 128 + 4*96 + 2*1]: identity | wq|wk|wv|wfc
    # (each [96,96] natural, zero-padded to 128 rows) | gamma | beta columns
    wall_d = nc.dram_tensor("wall", [P, P + 4 * D + 2], F32R,
                            kind="ExternalInput").ap()
    out_d = nc.dram_tensor("out", [C, N, D], F32, kind="ExternalOutput").ap()

    const = ctx.enter_context(tc.tile_pool(name="const", bufs=1))
    pc = ctx.enter_context(tc.tile_pool(name="perc", bufs=2))
    # 3-deep shared ring: score tiles + merged aux allocations (6 banks)
    w_psum = ctx.enter_context(tc.tile_pool(name="w_psum", bufs=3, space="PSUM"))
    ctx_psum = ctx.enter_context(tc.tile_pool(name="ctx_psum", bufs=1, space="PSUM"))

    wall = const.tile([P, P + 4 * D + 2], F32R)
    nc.sync.dma_start(out=wall, in_=wall_d)
    ident = wall[:, 0:P]

    # weights: transpose on PE; wq/wk stay f32r, wv/wfc cast to bf16
    wts = {}
    for k, nm in enumerate(("wq", "wk", "wv", "wfc")):
        w_nat = wall[0:D, P + k * D:P + (k + 1) * D]
        w_ps = w_psum.tile([D, D], F32R, name=f"{nm}_ps", tag="w")
        nc.tensor.transpose(w_ps, w_nat, ident[0:D, 0:D])
        dt = F32R if nm in ("wq", "wk") else BF16
        w_t = const.tile([D, D], dt, name=f"{nm}T", tag=f"{nm}T")
        nc.vector.tensor_copy(out=w_t, in_=w_ps)
        wts[nm] = w_t

    gam_tile = bet_tile = None
    if apply_affine:
        gam_tile = const.tile([P, D], F32)
        bet_tile = const.tile([P, D], F32)
        for t, col in ((gam_tile, P + 4 * D), (bet_tile, P + 4 * D + 1)):
            col_ap = wall_d[0:D, col:col + 1]
            bcast = bass.AP(tensor=col_ap.tensor, offset=col_ap.offset,
                            ap=[[0, P], col_ap.ap[0]])
            nc.gpsimd.dma_start(out=t, in_=bcast)

    st = {}

    def load_c(c, split=False):
        """DMA loads + per-channel tile allocation for channel c"""
        s = {}
        s["xq_nat"] = pc.tile([P, NCHUNK, D], F32R, name=f"xq{c}", tag="xq",
                              bufs=3)
        s["xk_nat"] = pc.tile([P, NCHUNK, D], F32R, name=f"xk{c}", tag="xk",
                              bufs=1)
        s["xv_nat"] = pc.tile([P, NCHUNK, D], F32R, name=f"xv{c}", tag="xv",
                              bufs=1)
        half = NCHUNK // 2
        if split:
            # prologue: interleave half-tensor DMAs across the SP and
            # Activation queues so both tensors' first halves land first
            for nm, src, g, eng in (("xq_nat", xq_d, 0, nc.sync),
                                    ("xk_nat", xk_d, 0, nc.scalar),
                                    ("xq_nat", xq_d, 1, nc.sync),
                                    ("xk_nat", xk_d, 1, nc.scalar)):
                eng.dma_start(
                    out=s[nm][:, g * half:(g + 1) * half, :],
                    in_=src[c, g * half * P:(g + 1) * half * P].rearrange(
                        "(i p) d -> p i d", p=P))
        else:
            nc.sync.dma_start(
                out=s["xq_nat"], in_=xq_d[c].rearrange("(i p) d -> p i d", p=P))
            nc.sync.dma_start(
                out=s["xk_nat"], in_=xk_d[c].rearrange("(i p) d -> p i d", p=P))
        nc.gpsimd.dma_start(
            out=s["xv_nat"], in_=xv_d[c].rearrange("(i p) d -> p i d", p=P))
        s["xqT"] = pc.tile([D, N], F32R, name=f"xqT{c}", tag="xqT", bufs=1)
        s["xkT"] = pc.tile([D, N], F32R, name=f"xkT{c}", tag="xkT", bufs=1)
        s["xvT"] = pc.tile([D, N], BF16, name=f"xvT{c}", tag="xvT", bufs=1)
        s["qdT"] = pc.tile([D, N], BF16, name=f"qdT{c}", tag="qdT")
        s["kdT"] = pc.tile([D, N], BF16, name=f"kdT{c}", tag="kdT")
        s["v_nat"] = pc.tile([P, NCHUNK, D], BF16, name=f"v{c}", tag="v")
        s["ssum"] = pc.tile([P, H * NCHUNK], F32, name=f"ssum{c}", tag="ssum")
        s["sinv"] = pc.tile([P, H * NCHUNK], F32, name=f"sinv{c}", tag="sinv")
        s["e_all"] = pc.tile([P, H * NCHUNK, N], BF16, name=f"e{c}", tag="e")
        s["vsc"] = pc.tile([P, H * NCHUNK, DV], BF16, name=f"vsc{c}", tag="vsc")
        s["s_tiles"] = {}
        st[c] = s

    # ---- merged aux ops: at most one ring allocation each ----
    def tr_all(c, nm, split=False):
        """PE-transpose all 8 chunks of one input -> X.T (one allocation)"""
        s = st[c]
        src = s[f"x{nm}_nat"]
        tp = w_psum.tile([D, N], F32R, name=f"tp{nm}{c}", tag="w")
        if split:
            for g in range(2):
                for j in range(4):
                    i = 4 * g + j
                    nc.tensor.transpose(tp[:, i * P:(i + 1) * P],
                                        src[:, i, :], ident)
                nc.vector.tensor_copy(
                    out=s[f"x{nm}T"][:, g * QT:(g + 1) * QT],
                    in_=tp[:, g * QT:(g + 1) * QT])
        else:
            for i in range(NCHUNK):
                nc.tensor.transpose(tp[:, i * P:(i + 1) * P], src[:, i, :],
                                    ident)
            nc.vector.tensor_copy(out=s[f"x{nm}T"], in_=tp)

    def proj_qk(c, which, split=False):
        """Q or K projection (f32r, free=512), both halves, one allocation"""
        s = st[c]
        nm, w_t = (("qdT", wts["wq"]) if which == "q" else ("kdT", wts["wk"]))
        pr = w_psum.tile([D, N], F32, name=f"pr{c}{which}", tag="w")
        for g in range(2):
            nc.tensor.matmul(pr[:, g * QT:(g + 1) * QT], lhsT=w_t,
                             rhs=s[f"x{which}T"][:, g * QT:(g + 1) * QT],
                             start=True, stop=True)
            if split:
                nc.vector.tensor_copy(
                    out=s[nm][:, g * QT:(g + 1) * QT],
                    in_=pr[:, g * QT:(g + 1) * QT])
        if not split:
            nc.vector.tensor_copy(out=s[nm], in_=pr)

    def proj_v(c):
        """V projection in bf16 (1 cycle/row), all 8 chunks, one allocation.
        [P, 2, QT] layout: 4 chunks per PSUM bank so no matmul output
        crosses a bank boundary."""
        s = st[c]
        vp = w_psum.tile([P, 2, QT], F32, name=f"vp{c}", tag="w")
        for i in range(NCHUNK):
            g, j = divmod(i, 4)
            nc.tensor.matmul(vp[:, g, j * D:(j + 1) * D],
                             lhsT=s["xvT"][:, i * P:(i + 1) * P],
                             rhs=wts["wv"], start=True, stop=True)
        nc.vector.tensor_copy(
            out=s["v_nat"].rearrange("p (g i) d -> p g (i d)", g=2),
            in_=vp[:, :, 0:4 * D])

    def scores(c, i):
        """S_T[k, q] for chunk i, h-outer (one kdT weight load per head)"""
        s = st[c]
        for h in range(H):
            hs = slice(DK * h, DK * (h + 1))
            s_t = w_psum.tile([P, N], F32, name=f"s{c}_{i}_{h}", tag="w")
            for g in range(2):
                nc.tensor.matmul(
                    s_t[:, g * QT:(g + 1) * QT],
                    lhsT=s["kdT"][hs, i * P:(i + 1) * P],
                    rhs=s["qdT"][hs, g * QT:(g + 1) * QT],
                    start=True, stop=True)
            s["s_tiles"][(i, h)] = s_t

    def exp_slot(c, i):
        """exp on ScalarE with fused 1/sqrt(dk) scale; accum -> denominators"""
        s = st[c]
        for h in range(H):
            j = i * H + h
            nc.scalar.activation(
                out=s["e_all"][:, j, :], in_=s["s_tiles"].pop((i, h)),
                func=mybir.ActivationFunctionType.Exp,
                scale=SCALE, accum_out=s["ssum"][:, j:j + 1])

    def recip_vsc(c, i):
        """denominator reciprocal + V' = V/denom for chunk i (DVE).
        Emitted at the TOP of slot i+1 (before the slot's casts) so ctx
        never waits behind PE-gated copy work on the in-order DVE queue."""
        s = st[c]
        nc.vector.reciprocal(out=s["sinv"][:, i * H:(i + 1) * H],
                             in_=s["ssum"][:, i * H:(i + 1) * H])
        for h in range(H):
            hs = slice(DK * h, DK * (h + 1))
            j = i * H + h
            nc.vector.tensor_scalar_mul(
                out=s["vsc"][:, j, :], in0=s["v_nat"][:, i, hs],
                scalar1=s["sinv"][:, j:j + 1])

    def ctx_mm(c, i):
        """context accumulation for chunk i (bf16, h-outer)"""
        s = st[c]
        for h in range(H):
            hs = slice(DV * h, DV * (h + 1))
            j = i * H + h
            for g in range(2):
                nc.tensor.matmul(
                    s["ctx_ps"][hs, g * QT:(g + 1) * QT],
                    lhsT=s["vsc"][:, j, :],
                    rhs=s["e_all"][:, j, g * QT:(g + 1) * QT],
                    start=(i == 0), stop=(i == NCHUNK - 1),
                    skip_group_check=True)

    # ---- tail pieces, parameterized by chunk range [lo, hi) ----
    def tail_a(c, lo=0, hi=NCHUNK):
        """ctx copy-out to bf16 (frees the ctx PSUM banks)"""
        s = st[c]
        if "ctxT" not in s:
            s["ctxT"] = pc.tile([D, N], BF16, name=f"ctxT{c}", tag="ctxT",
                                bufs=1)
        nc.vector.tensor_copy(out=s["ctxT"][:, lo * P:hi * P],
                              in_=s["ctx_ps"][:, lo * P:hi * P])
        if hi == NCHUNK:
            del s["ctx_ps"]

    def tail_fc(c, lo=0, hi=NCHUNK):
        """fc matmuls (bf16); 4 chunks per PSUM bank"""
        s = st[c]
        if "fc_ps" not in s:
            s["fc_ps"] = w_psum.tile([P, 2, QT], F32, name=f"fc{c}", tag="w")
        for i in range(lo, hi):
            g, j = divmod(i, 4)
            nc.tensor.matmul(s["fc_ps"][:, g, j * D:(j + 1) * D],
                             lhsT=s["ctxT"][:, i * P:(i + 1) * P],
                             rhs=wts["wfc"], start=True, stop=True)

    def tail_sums(c, lo=0, hi=NCHUNK):
        """residual add + row-sum accumulation (DVE, reads fc from PSUM)"""
        s = st[c]
        if "t_all" not in s:
            s["t_all"] = pc.tile([P, NCHUNK, D], F32, name=f"t{c}", tag="t",
                                 bufs=1)
            s["sums"] = pc.tile([P, NCHUNK], F32, name=f"sm{c}", tag="sm",
                                bufs=1)
        for i in range(lo, hi):
            g, j = divmod(i, 4)
            nc.vector.scalar_tensor_tensor(
                out=s["t_all"][:, i, :], in0=s["fc_ps"][:, g, j * D:(j + 1) * D],
                scalar=1.0, in1=s["xq_nat"][:, i, :].bitcast(F32),
                op0=A.mult, op1=A.add, accum_out=s["sums"][:, i:i + 1])
        if hi == NCHUNK:
            del s["fc_ps"]

    def tail_sumsq(c, lo=0, hi=NCHUNK):
        s = st[c]
        if "sumsq" not in s:
            s["sumsq"] = pc.tile([P, NCHUNK], F32, name=f"sq{c}", tag="sq",
                                 bufs=1)
            s["scr"] = pc.tile([P, NCHUNK, D], F32, name=f"scr{c}", tag="scr",
                               bufs=1)
        for i in range(lo, hi):
            nc.vector.scalar_tensor_tensor(
                out=s["scr"][:, i, :], in0=s["t_all"][:, i, :], scalar=1.0,
                in1=s["t_all"][:, i, :], op0=A.mult, op1=A.mult,
                accum_out=s["sumsq"][:, i:i + 1])

    def tail_ln1(c, lo=0, hi=NCHUNK):
        """mean, var, rstd via bit-hack + 2 Newton iterations (DVE only)"""
        s = st[c]
        if "mean" not in s:
            s["mean"] = pc.tile([P, NCHUNK], F32, name=f"mean{c}", tag="mean",
                                bufs=1)
            s["msq"] = pc.tile([P, NCHUNK], F32, name=f"msq{c}", tag="msq",
                               bufs=1)
            s["var"] = pc.tile([P, NCHUNK], F32, name=f"var{c}", tag="var",
                               bufs=1)
            s["y"] = pc.tile([P, NCHUNK], F32, name=f"y{c}", tag="y", bufs=1)
            s["t1"] = pc.tile([P, NCHUNK], F32, name=f"t1{c}", tag="t1",
                              bufs=1)
        sl = slice(lo, hi)
        mean, msq, var = s["mean"][:, sl], s["msq"][:, sl], s["var"][:, sl]
        y, t1 = s["y"][:, sl], s["t1"][:, sl]
        nc.vector.tensor_scalar_mul(out=mean, in0=s["sums"][:, sl],
                                    scalar1=1.0 / D)
        nc.vector.tensor_mul(out=msq, in0=mean, in1=mean)
        nc.vector.scalar_tensor_tensor(
            out=var, in0=s["sumsq"][:, sl], scalar=1.0 / D, in1=msq,
            op0=A.mult, op1=A.subtract)
        nc.vector.tensor_scalar_add(out=var, in0=var, scalar1=EPS)
        nc.vector.tensor_scalar(
            out=y.bitcast(I32), in0=var.bitcast(I32), scalar1=1,
            scalar2=None, op0=A.logical_shift_right)
        nc.vector.tensor_scalar(
            out=y.bitcast(I32), in0=y.bitcast(I32), scalar1=-1,
            scalar2=None, op0=A.bitwise_xor)
        nc.vector.tensor_scalar(
            out=y.bitcast(I32), in0=y.bitcast(I32), scalar1=0x5F3759E0,
            scalar2=None, op0=A.add)
        for _ in range(2):
            nc.vector.tensor_mul(out=t1, in0=y, in1=y)
            nc.vector.tensor_mul(out=t1, in0=t1, in1=var)
            nc.vector.tensor_scalar(out=t1, in0=t1, scalar1=-0.5, scalar2=1.5,
                                    op0=A.mult, op1=A.add)
            nc.vector.tensor_mul(out=y, in0=y, in1=t1)

    def tail_ln2(c, lo=0, hi=NCHUNK):
        """normalize + (affine) + store"""
        s = st[c]
        if "osb" not in s:
            s["osb"] = pc.tile([P, NCHUNK, D], F32, name=f"osb{c}", tag="osb",
                               bufs=1)
        out_sb = s["osb"]
        for i in range(lo, hi):
            nc.vector.tensor_scalar(
                out=out_sb[:, i, :], in0=s["t_all"][:, i, :],
                scalar1=s["mean"][:, i:i + 1], scalar2=s["y"][:, i:i + 1],
                op0=A.subtract, op1=A.mult)
        if apply_affine:
            for i in range(lo, hi):
                nc.vector.tensor_mul(out=out_sb[:, i, :], in0=out_sb[:, i, :],
                                     in1=gam_tile)
                nc.vector.tensor_add(out=out_sb[:, i, :], in0=out_sb[:, i, :],
                                     in1=bet_tile)
        nc.sync.dma_start(
            out=out_d[c, lo * P:hi * P].rearrange("(i p) d -> p i d", p=P),
            in_=out_sb[:, lo:hi, :])

    # ---------------- prologue: phase-1 of channel 0 ----------------
    load_c(0, split=True)
    tr_all(0, "q", split=True)
    proj_qk(0, "q", split=True)
    tr_all(0, "k", split=True)
    proj_qk(0, "k", split=True)
    scores(0, 0)
    tr_all(0, "v")   # V path runs inside the exp(0,0) block's shadow
    proj_v(0)

    # ---------------- slot loop ----------------
    # per slot i: ScalarE exp(c,i) | DVE: recip+vsc(i-1) first, aux copies
    # + one tail piece | PE: ctx(i-1), at most one merged aux allocation,
    # scores(i+1) last
    for c in range(C):
        st[c]["ctx_ps"] = ctx_psum.tile([D, N], F32, name=f"ctx{c}", tag="ctx")
        for i in range(NCHUNK):
            exp_slot(c, i)
            # denominators for the previous chunk (accums landed last slot)
            if i >= 1:
                recip_vsc(c, i - 1)
            elif c >= 1:
                recip_vsc(c - 1, NCHUNK - 1)
            # ---- PE: ctx first (vsc just computed), then aux, then scores
            if i >= 1:
                ctx_mm(c, i - 1)
            elif c >= 1:
                ctx_mm(c - 1, NCHUNK - 1)
            # ---- slot's aux piece + tail piece + DMA kicks ----
            if i == 0:
                if c >= 1:
                    tail_a(c - 1)
                if c + 1 < C:
                    load_c(c + 1)
            elif i == 1:
                if c >= 1:
                    tail_fc(c - 1)
                    tail_sums(c - 1)
            elif i == 2:
                if c + 1 < C:
                    tr_all(c + 1, "q")
                if c >= 1:
                    tail_sumsq(c - 1)
            elif i == 3:
                if c + 1 < C:
                    proj_qk(c + 1, "q")
                if c >= 1:
                    tail_ln1(c - 1)
            elif i == 4:
                if c + 1 < C:
                    tr_all(c + 1, "k")
                if c >= 1:
                    tail_ln2(c - 1)
            elif i == 5:
                if c + 1 < C:
                    proj_qk(c + 1, "k")
            elif i == 6:
                if c + 1 < C:
                    tr_all(c + 1, "v")
            elif i == 7:
                if c + 1 < C:
                    proj_v(c + 1)
            # ---- next scores last (self-paced by the ring) ----
            if i < NCHUNK - 1:
                scores(c, i + 1)
            elif c + 1 < C:
                scores(c + 1, 0)

    # ---------------- drain: last channel tail, half-pipelined ----------------
    cl = C - 1
    recip_vsc(cl, NCHUNK - 1)
    ctx_mm(cl, NCHUNK - 1)
    tail_a(cl, 0, 4)
    tail_fc(cl, 0, 4)
    tail_a(cl, 4, 8)
    tail_sums(cl, 0, 4)
    tail_fc(cl, 4, 8)
    tail_sumsq(cl, 0, 4)
    tail_sums(cl, 4, 8)
    tail_ln1(cl, 0, 4)
    tail_sumsq(cl, 4, 8)
    tail_ln2(cl, 0, 4)
    tail_ln1(cl, 4, 8)
    tail_ln2(cl, 4, 8)


def _build(apply_affine):
    nc = bacc.Bacc("TRN2", target_bir_lowering=False, debug=False, num_devices=B)
    with tile.TileContext(nc) as tc, ExitStack() as ctx:
        _emit(nc, tc, ctx, apply_affine)
    nc.compile()
    return nc


def _prep_in_maps(input_Q, input_K, input_V, W_Q, W_K, W_V, W_fc,
                  ln_gamma, ln_beta):
    """Host-side packing of the constant wall + per-core input maps."""
    wall = np.zeros((P, P + 4 * D + 2), dtype=np.float32)
    wall[:, :P] = np.eye(P, dtype=np.float32)
    for k, W in enumerate((W_Q, W_K, W_V, W_fc)):
        wall[0:D, P + k * D:P + (k + 1) * D] = W
    wall[0:D, P + 4 * D] = ln_gamma
    wall[0:D, P + 4 * D + 1] = ln_beta
    return [
        {"xq": input_Q[b], "xk": input_K[b], "xv": input_V[b], "wall": wall}
        for b in range(B)
    ]


def kernel(input_Q, input_K, input_V, W_Q, W_K, W_V, W_fc, ln_gamma, ln_beta):
    input_Q = np.ascontiguousarray(np.asarray(input_Q, dtype=np.float32))
    input_K = np.ascontiguousarray(np.asarray(input_K, dtype=np.float32))
    input_V = np.ascontiguousarray(np.asarray(input_V, dtype=np.float32))
    W_Q = np.ascontiguousarray(np.asarray(W_Q, dtype=np.float32))
    W_K = np.ascontiguousarray(np.asarray(W_K, dtype=np.float32))
    W_V = np.ascontiguousarray(np.asarray(W_V, dtype=np.float32))
    W_fc = np.ascontiguousarray(np.asarray(W_fc, dtype=np.float32))
    ln_gamma = np.ascontiguousarray(np.asarray(ln_gamma, dtype=np.float32))
    ln_beta = np.ascontiguousarray(np.asarray(ln_beta, dtype=np.float32))

    apply_affine = not (np.all(ln_gamma == 1.0) and np.all(ln_beta == 0.0))

    key = ("nc", apply_affine)
    if key not in _CACHE:
        _CACHE[key] = _build(apply_affine)
    nc = _CACHE[key]

    in_maps = _prep_in_maps(input_Q, input_K, input_V, W_Q, W_K, W_V, W_fc,
                            ln_gamma, ln_beta)
    res = run_bass_kernel_spmd(nc, in_maps, core_ids=list(range(B)))
    return np.stack([res.results[b]["out"] for b in range(B)], axis=0)
